# revision 7
# baseline (speedup 1.0000x reference)
"""Trainium2 Bass kernel for nn_BiLSTM_24567212934024.

Reference semantics (note the source bugs in the reference module):
  - Both scan directions use the FORWARD cell weights.
  - Per-step outputs come only from the forward scan; out_b = out_f flipped on batch.
  - The backward scan only contributes its final (h, c) state.

Strategy (8 NeuronCores, data parallel, no collectives):
  - Core c owns batch rows 8c..8c+8. Its scan state has 16 rows:
    rows 0-7  = forward chain (consumes x[:, t]),
    rows 8-15 = backward chain (consumes x[:, 511-t]); both share Wih/Whh.
  - Phase 1 (precompute): xW = x @ Wih.T for all timesteps as one large fp16
    matmul (fp32 PSUM accumulate), written to DRAM.
  - Phase 2 (scan): per step, gates = h @ Whh.T + xW_t + b using 4-way
    column-tiled fp16 matmuls (PE tile_position), identity-injection matmul to
    add xW_t + b, then sigmoid/tanh + c/h update on 112 partitions, then two
    PE transposes produce the next step's transposed-h stationary (fp16).

Layout: gate columns are permuted so column group j (psum partitions
32j..32j+16) holds [i|f|o|g] x 256 for h-slice [256j:256j+256]; all
elementwise ops are partition-aligned.
"""
import os
import sys

for _p in ("/opt/trn_rl_repo", "/root/.axon_site/_ro/trn_rl_repo"):
    if os.path.isdir(_p) and _p not in sys.path:
        sys.path.insert(0, _p)

import numpy as np
import concourse.mybir as mybir
import concourse.tile as tile
from concourse import bacc
from concourse.bass_utils import run_bass_kernel_spmd

F32 = mybir.dt.float32
F16 = mybir.dt.float16

B, T, I, H = 64, 512, 1024, 1024
NCORES = 8
RPC = B // NCORES          # batch rows per core = 8
M = 2 * RPC                # scan state rows per core = 16
GQ = H // 4                # h-cols per col group = 256
NK = H // 128              # K chunks = 8
NSEG = 2                   # 512-col psum segments per group
ACT_P = 112                # partitions spanned by elementwise ops

# gate permutation: reference gate order along 4H is [i, f, g, o].
# perm[j*1024 + slot*256 + s] = src column, slot order [i, f, o, g].
_GATE_SRC = [0, 1, 3, 2]   # i, f, o, g -> position in reference order
PERM = np.zeros(4 * H, dtype=np.int64)
for _j in range(4):
    for _slot, _src in enumerate(_GATE_SRC):
        PERM[_j * H + _slot * GQ:(_j) * H + (_slot + 1) * GQ] = \
            np.arange(_src * H + _j * GQ, _src * H + (_j + 1) * GQ)

_NC_CACHE = {}
LAST_RESULTS = None        # test harness reads exec_time from here


def _build(t_steps: int):
    nc = bacc.Bacc(None, target_bir_lowering=False)
    R = RPC * t_steps      # precompute row count

    d_xT = nc.dram_tensor("xT", [I, R], F16, kind="ExternalInput")
    d_Wt = nc.dram_tensor("Wt", [I, 4 * H], F16, kind="ExternalInput")
    d_WhhT = nc.dram_tensor("WhhT", [H, 4 * H], F16, kind="ExternalInput")
    d_bias = nc.dram_tensor("bias", [1, 4 * H], F16, kind="ExternalInput")
    d_inj = nc.dram_tensor("inj", [128, M], F16, kind="ExternalInput")
    d_id = nc.dram_tensor("idm", [ACT_P, ACT_P], F32, kind="ExternalInput")
    d_h0T = nc.dram_tensor("h0T", [128, 2, 128], F16, kind="ExternalInput")
    d_c0 = nc.dram_tensor("c0", [128, GQ], F32, kind="ExternalInput")

    d_hout = nc.dram_tensor("hout", [t_steps, 4, RPC, GQ], F32, kind="ExternalOutput")
    d_hfin = nc.dram_tensor("hfin", [128, GQ], F32, kind="ExternalOutput")
    d_cfin = nc.dram_tensor("cfin", [128, GQ], F32, kind="ExternalOutput")

    n_mtiles = R // 128

    with tile.TileContext(nc) as tc:
        with tc.tile_pool(name="dram", bufs=1, space="DRAM") as dpool:
            d_xw = dpool.tile([R, 4 * H], F16, tag="xw")

            # ---------------- phase 1: xW = x @ Wih.T (+0) ----------------
            with tc.tile_pool(name="wt", bufs=1) as wtp, \
                 tc.tile_pool(name="xt", bufs=4) as xtp, \
                 tc.tile_pool(name="xwev", bufs=2) as xwe, \
                 tc.tile_pool(name="pps", bufs=2, space="PSUM") as pps:
                t_Wt = wtp.tile([128, NK, 4 * H], F16, tag="Wt")
                nc.sync.dma_start(out=t_Wt, in_=d_Wt[:].rearrange("(k p) n -> p k n", p=128))
                for mt in range(n_mtiles):
                    xts = []
                    for k in range(NK):
                        t_x = xtp.tile([128, 128], F16, tag=f"xt{k % 4}")
                        nc.sync.dma_start(
                            out=t_x,
                            in_=d_xT[k * 128:(k + 1) * 128, mt * 128:(mt + 1) * 128])
                        xts.append(t_x)
                    t_ev = xwe.tile([128, 4 * H], F16, tag="ev")
                    for seg in range(8):
                        pp = pps.tile([128, 512], F32, tag="pp")
                        for k in range(NK):
                            nc.tensor.matmul(
                                out=pp,
                                lhsT=xts[k],
                                rhs=t_Wt[:, k, seg * 512:(seg + 1) * 512],
                                start=(k == 0), stop=(k == NK - 1))
                        nc.vector.tensor_copy(out=t_ev[:, seg * 512:(seg + 1) * 512], in_=pp)
                    nc.sync.dma_start(out=d_xw[mt * 128:(mt + 1) * 128, :], in_=t_ev)

            # ---------------- phase 2: recurrent scan ----------------
            with tc.tile_pool(name="whh", bufs=1) as whp, \
                 tc.tile_pool(name="state", bufs=1) as stp, \
                 tc.tile_pool(name="work", bufs=2) as wkp, \
                 tc.tile_pool(name="gps", bufs=2, space="PSUM") as gpsp, \
                 tc.tile_pool(name="tps", bufs=2, space="PSUM") as tpsp:

                t_W = whp.tile([128, NK, 4 * H], F16, tag="Whh")
                nc.sync.dma_start(out=t_W, in_=d_WhhT[:].rearrange("(k p) n -> p k n", p=128))
                t_inj = stp.tile([128, M], F16, tag="inj")
                nc.sync.dma_start(out=t_inj, in_=d_inj[:])
                t_id = stp.tile([ACT_P, ACT_P], F32, tag="idm")
                nc.sync.dma_start(out=t_id, in_=d_id[:])

                # persistent ring tiles
                NXB = 3
                xwb = [stp.tile([128, 4 * H], F16, tag=f"xwb{i}", name=f"xwb{i}") for i in range(NXB)]
                hT2 = [[stp.tile([128, 128], F16, tag=f"hT{i}s{s}", name=f"hT{i}s{s}")
                        for s in range(2)] for i in range(2)]
                c_r = [stp.tile([128, GQ], F32, tag=f"c{i}", name=f"c{i}") for i in range(2)]
                for i in range(NXB):
                    nc.vector.memset(xwb[i], 0.0)
                    nc.sync.dma_start(out=xwb[i][16:17, :], in_=d_bias[:])
                for s in range(2):
                    nc.sync.dma_start(out=hT2[1][s], in_=d_h0T[:, s, :])
                nc.sync.dma_start(out=c_r[1], in_=d_c0[:])

                xw_r = d_xw[:].rearrange("(r t) n -> t r n", t=t_steps)

                for t in range(t_steps):
                    cur, prv = t % 2, (t + 1) % 2
                    xcur = t % NXB
                    # per-step xW rows: fwd at t (rows 0-7), bwd at T-1-t (rows 8-15)
                    nc.sync.dma_start(out=xwb[xcur][0:RPC, :], in_=xw_r[t])
                    nc.sync.dma_start(out=xwb[xcur][RPC:M, :], in_=xw_r[t_steps - 1 - t])

                    g_ps = gpsp.tile([128, 4 * H // 4], F32, tag="g")  # [128, 1024]
                    if t < 2:
                        nc.vector.memset(g_ps, 0.0)  # keep junk partitions finite
                    # chunk order: even chunks (hT half 0) first so the next
                    # step can begin once half 0 of h is transposed
                    K_ORDER = [0, 2, 4, 6, 1, 3, 5, 7]
                    for seg in range(NSEG):
                        cs = slice(seg * 512, (seg + 1) * 512)
                        # interleave col groups so the 4 PE column tiles
                        # stream concurrently; injection first (no dependency
                        # on this step's transposed h)
                        for j in range(4):
                            wcols = slice(j * H + seg * 512, j * H + (seg + 1) * 512)
                            nc.tensor.matmul(
                                out=g_ps[32 * j:32 * j + M, cs],
                                lhsT=t_inj, rhs=xwb[xcur][:, wcols],
                                start=True, stop=False, tile_position=(0, 32 * j))
                        for ki, k in enumerate(K_ORDER):
                            for j in range(4):
                                wcols = slice(j * H + seg * 512, j * H + (seg + 1) * 512)
                                nc.tensor.matmul(
                                    out=g_ps[32 * j:32 * j + M, cs],
                                    lhsT=hT2[prv][k % 2][:, 32 * (k // 2):32 * (k // 2) + M],
                                    rhs=t_W[:, k, wcols],
                                    start=False, stop=(ki == NK - 1),
                                    tile_position=(0, 32 * j))
                        if seg == 0:
                            t_act = wkp.tile([128, 4 * H // 4], F32, tag="act")
                            nc.scalar.activation(
                                out=t_act[0:ACT_P, 0:512], in_=g_ps[0:ACT_P, 0:512],
                                func=mybir.ActivationFunctionType.Sigmoid)
                    nc.scalar.activation(
                        out=t_act[0:ACT_P, 512:768], in_=g_ps[0:ACT_P, 512:768],
                        func=mybir.ActivationFunctionType.Sigmoid)
                    nc.scalar.activation(
                        out=t_act[0:ACT_P, 768:1024], in_=g_ps[0:ACT_P, 768:1024],
                        func=mybir.ActivationFunctionType.Tanh)

                    t_m1 = wkp.tile([128, GQ], F32, tag="m1")
                    t_m2 = wkp.tile([128, GQ], F32, tag="m2")
                    t_tc = wkp.tile([128, GQ], F32, tag="tc")
                    t_h = wkp.tile([128, GQ], F32, tag="h")
                    a = t_act[0:ACT_P]
                    for s in range(2):
                        hs = slice(s * 128, (s + 1) * 128)
                        nc.vector.tensor_mul(out=t_m1[0:ACT_P, hs],
                                             in0=a[:, s * 128:(s + 1) * 128],
                                             in1=a[:, 3 * GQ + s * 128:3 * GQ + (s + 1) * 128])
                        nc.vector.tensor_mul(out=t_m2[0:ACT_P, hs],
                                             in0=a[:, GQ + s * 128:GQ + (s + 1) * 128],
                                             in1=c_r[prv][0:ACT_P, hs])
                        nc.vector.tensor_add(out=c_r[cur][0:ACT_P, hs],
                                             in0=t_m1[0:ACT_P, hs], in1=t_m2[0:ACT_P, hs])
                        nc.scalar.activation(out=t_tc[0:ACT_P, hs], in_=c_r[cur][0:ACT_P, hs],
                                             func=mybir.ActivationFunctionType.Tanh)
                        nc.vector.tensor_mul(out=t_h[0:ACT_P, hs],
                                             in0=a[:, 2 * GQ + s * 128:2 * GQ + (s + 1) * 128],
                                             in1=t_tc[0:ACT_P, hs])
                        tp_ps = tpsp.tile([128, 128], F32, tag=f"tp{s}", name=f"tp{s}")
                        nc.tensor.transpose(
                            out=tp_ps[0:128, 0:ACT_P],
                            in_=t_h[0:ACT_P, hs],
                            identity=t_id)
                        nc.vector.tensor_copy(out=hT2[cur][s][:, 0:ACT_P],
                                              in_=tp_ps[:, 0:ACT_P])

                    # store forward rows (state rows 0-7 of each group)
                    for j in range(4):
                        nc.sync.dma_start(
                            out=d_hout[t, j],
                            in_=t_h[32 * j:32 * j + RPC, :])

                    if t == t_steps - 1:
                        nc.sync.dma_start(out=d_hfin[:], in_=t_h)
                        nc.sync.dma_start(out=d_cfin[:], in_=c_r[cur])
    nc.finalize()
    return nc


def _get_nc(t_steps: int):
    if t_steps not in _NC_CACHE:
        _NC_CACHE[t_steps] = _build(t_steps)
    return _NC_CACHE[t_steps]


def _prep_core_inputs(c, x, Wt_f16, WhhT_f16, bias_f16, inj, idm, h0f, c0f, h0b, c0b,
                      t_steps):
    rows = slice(RPC * c, RPC * (c + 1))
    x_c = x[rows, :t_steps, :]                                # [8, T, I]
    xT = np.ascontiguousarray(
        x_c.reshape(RPC * t_steps, I).T).astype(np.float16)   # [I, R]

    hstate = np.concatenate([h0f[rows], h0b[rows]], axis=0)   # [16, H]
    cstate = np.concatenate([c0f[rows], c0b[rows]], axis=0)
    h0T = np.zeros((128, 2, 128), np.float16)
    c0 = np.zeros((128, GQ), np.float32)
    for j in range(4):
        for s in range(2):
            h0T[:, s, 32 * j:32 * j + M] = \
                hstate[:, 256 * j + 128 * s:256 * j + 128 * (s + 1)].T
        c0[32 * j:32 * j + M, :] = cstate[:, GQ * j:GQ * (j + 1)]
    return {"xT": xT, "Wt": Wt_f16, "WhhT": WhhT_f16, "bias": bias_f16,
            "inj": inj, "idm": idm, "h0T": h0T, "c0": c0}


def kernel(x, Wih_f, Whh_f, bih_f, bhh_f, Wih_b, Whh_b, bih_b, bhh_b,
           h0f, c0f, h0b, c0b, t_steps=T, trace=False):
    global LAST_RESULTS
    x = np.asarray(x, np.float32)
    Wih_f = np.asarray(Wih_f, np.float32)
    Whh_f = np.asarray(Whh_f, np.float32)
    bias = np.asarray(bih_f, np.float32) + np.asarray(bhh_f, np.float32)
    h0f = np.asarray(h0f, np.float32); c0f = np.asarray(c0f, np.float32)
    h0b = np.asarray(h0b, np.float32); c0b = np.asarray(c0b, np.float32)

    Wt_f16 = np.ascontiguousarray(Wih_f.T[:, PERM]).astype(np.float16)
    WhhT_f16 = np.ascontiguousarray(Whh_f.T[:, PERM]).astype(np.float16)
    bias_f16 = bias[PERM].astype(np.float16).reshape(1, 4 * H)
    inj = np.zeros((128, M), np.float16)
    for i in range(M):
        inj[i, i] = 1.0
    inj[16, :] = 1.0
    idm = np.eye(ACT_P, dtype=np.float32)

    nc = _get_nc(t_steps)
    in_maps = [
        _prep_core_inputs(c, x, Wt_f16, WhhT_f16, bias_f16, inj, idm,
                          h0f, c0f, h0b, c0b, t_steps)
        for c in range(NCORES)
    ]
    res = run_bass_kernel_spmd(nc, in_maps, core_ids=list(range(NCORES)),
                               trace=trace)
    LAST_RESULTS = res

    out_f = np.empty((B, t_steps, H), np.float32)
    hf = np.empty((B, H), np.float32); hb = np.empty((B, H), np.float32)
    cf = np.empty((B, H), np.float32); cb = np.empty((B, H), np.float32)
    for c in range(NCORES):
        rows = slice(RPC * c, RPC * (c + 1))
        ho = res.results[c]["hout"]                    # [T, 4, 8, 256]
        out_f[rows] = ho.transpose(2, 0, 1, 3).reshape(RPC, t_steps, H)
        hfin = res.results[c]["hfin"].reshape(4, 32, GQ)
        cfin = res.results[c]["cfin"].reshape(4, 32, GQ)
        hstate = hfin[:, :M, :].transpose(1, 0, 2).reshape(M, H)
        cstate = cfin[:, :M, :].transpose(1, 0, 2).reshape(M, H)
        hf[rows] = hstate[:RPC]; hb[rows] = hstate[RPC:]
        cf[rows] = cstate[:RPC]; cb[rows] = cstate[RPC:]

    output = np.concatenate([out_f, out_f[::-1]], axis=-1)
    h_i = np.concatenate([hf, hb], axis=-1)
    c_i = np.concatenate([cf, cb], axis=-1)
    return output, (h_i, c_i)


# revision 10
# speedup vs baseline: 1.0439x; 1.0439x over previous
"""Trainium2 Bass kernel for nn_BiLSTM_24567212934024.

Reference semantics (note the source bugs in the reference module):
  - Both scan directions use the FORWARD cell weights.
  - Per-step outputs come only from the forward scan; out_b = out_f flipped on batch.
  - The backward scan only contributes its final (h, c) state.

Strategy (8 NeuronCores, data parallel, no collectives):
  - Core c owns batch rows 8c..8c+8. Its scan state has 16 rows:
    rows 0-7  = forward chain (consumes x[:, t]),
    rows 8-15 = backward chain (consumes x[:, 511-t]); both share Wih/Whh.
  - Phase 1 (precompute): xW = x @ Wih.T for all timesteps as one large fp16
    matmul (fp32 PSUM accumulate), written to DRAM.
  - Phase 2 (scan): per step, gates = h @ Whh.T + xW_t + b using 4-way
    column-tiled fp16 matmuls (PE tile_position), identity-injection matmul to
    add xW_t + b, then sigmoid/tanh + c/h update on 112 partitions, then two
    PE transposes produce the next step's transposed-h stationary (fp16).

Layout: gate columns are permuted so column group j (psum partitions
32j..32j+16) holds [i|f|o|g] x 256 for h-slice [256j:256j+256]; all
elementwise ops are partition-aligned.
"""
import os
import sys

for _p in ("/opt/trn_rl_repo", "/root/.axon_site/_ro/trn_rl_repo"):
    if os.path.isdir(_p) and _p not in sys.path:
        sys.path.insert(0, _p)

import numpy as np
import concourse.mybir as mybir
import concourse.tile as tile
from concourse import bacc
from concourse.bass_utils import run_bass_kernel_spmd

F32 = mybir.dt.float32
F16 = mybir.dt.float16

B, T, I, H = 64, 512, 1024, 1024
NCORES = 8
RPC = B // NCORES          # batch rows per core = 8
M = 2 * RPC                # scan state rows per core = 16
GQ = H // 4                # h-cols per col group = 256
NK = H // 128              # K chunks = 8
NSEG = 2                   # 512-col psum segments per group
ACT_P = 112                # partitions spanned by elementwise ops

# gate permutation: reference gate order along 4H is [i, f, g, o].
# perm[j*1024 + slot*256 + s] = src column, slot order [i, f, o, g].
_GATE_SRC = [0, 1, 3, 2]   # i, f, o, g -> position in reference order
PERM = np.zeros(4 * H, dtype=np.int64)
for _j in range(4):
    for _slot, _src in enumerate(_GATE_SRC):
        PERM[_j * H + _slot * GQ:(_j) * H + (_slot + 1) * GQ] = \
            np.arange(_src * H + _j * GQ, _src * H + (_j + 1) * GQ)

_NC_CACHE = {}
LAST_RESULTS = None        # test harness reads exec_time from here


def _build(t_steps: int):
    nc = bacc.Bacc(None, target_bir_lowering=False)
    R = RPC * t_steps      # precompute row count

    d_xT = nc.dram_tensor("xT", [I, R], F16, kind="ExternalInput")
    d_Wt = nc.dram_tensor("Wt", [I, 4 * H], F16, kind="ExternalInput")
    d_WhhT = nc.dram_tensor("WhhT", [H, 4 * H], F16, kind="ExternalInput")
    d_bias = nc.dram_tensor("bias", [1, 4 * H], F32, kind="ExternalInput")
    d_id = nc.dram_tensor("idm", [ACT_P, ACT_P], F32, kind="ExternalInput")
    d_h0T = nc.dram_tensor("h0T", [128, 2, 128], F16, kind="ExternalInput")
    d_c0 = nc.dram_tensor("c0", [128, GQ], F32, kind="ExternalInput")

    d_hout = nc.dram_tensor("hout", [t_steps, 4, RPC, GQ], F32, kind="ExternalOutput")
    d_hfin = nc.dram_tensor("hfin", [128, GQ], F32, kind="ExternalOutput")
    d_cfin = nc.dram_tensor("cfin", [128, GQ], F32, kind="ExternalOutput")

    n_mtiles = R // 128

    with tile.TileContext(nc) as tc:
        with tc.tile_pool(name="dram", bufs=1, space="DRAM") as dpool:
            d_xw = dpool.tile([t_steps, 4, RPC, H], F32, tag="xw")

            # ---------------- phase 1: xW = x @ Wih.T (+0) ----------------
            with tc.tile_pool(name="wt", bufs=1) as wtp, \
                 tc.tile_pool(name="xt", bufs=4) as xtp, \
                 tc.tile_pool(name="xwev", bufs=2) as xwe, \
                 tc.tile_pool(name="pps", bufs=2, space="PSUM") as pps:
                t_Wt = wtp.tile([128, NK, 4 * H], F16, tag="Wt")
                nc.sync.dma_start(out=t_Wt, in_=d_Wt[:].rearrange("(k p) n -> p k n", p=128))
                t_bb = wtp.tile([128, 4 * H], F32, tag="biasbc")
                nc.sync.dma_start(out=t_bb, in_=d_bias[:].to_broadcast((128, 4 * H)))
                for mt in range(n_mtiles):
                    xts = []
                    for k in range(NK):
                        t_x = xtp.tile([128, 128], F16, tag=f"xt{k % 4}")
                        nc.sync.dma_start(
                            out=t_x,
                            in_=d_xT[k * 128:(k + 1) * 128, mt * 128:(mt + 1) * 128])
                        xts.append(t_x)
                    t_ev = xwe.tile([128, 4 * H], F32, tag="ev")
                    for seg in range(8):
                        pp = pps.tile([128, 512], F32, tag="pp")
                        for k in range(NK):
                            nc.tensor.matmul(
                                out=pp,
                                lhsT=xts[k],
                                rhs=t_Wt[:, k, seg * 512:(seg + 1) * 512],
                                start=(k == 0), stop=(k == NK - 1))
                        nc.vector.tensor_add(out=t_ev[:, seg * 512:(seg + 1) * 512],
                                             in0=pp, in1=t_bb[:, seg * 512:(seg + 1) * 512])
                    p = 0
                    while p < 128:
                        rr = (mt * 128 + p) // t_steps
                        tt = (mt * 128 + p) % t_steps
                        run = min(128 - p, t_steps - tt)
                        for j in range(4):
                            nc.sync.dma_start(
                                out=d_xw[tt:tt + run, j, rr, :],
                                in_=t_ev[p:p + run, j * H:(j + 1) * H])
                        p += run

            # ---------------- phase 2: recurrent scan ----------------
            with tc.tile_pool(name="whh", bufs=1) as whp, \
                 tc.tile_pool(name="state", bufs=1) as stp, \
                 tc.tile_pool(name="work", bufs=2) as wkp, \
                 tc.tile_pool(name="gps", bufs=2, space="PSUM") as gpsp, \
                 tc.tile_pool(name="tps", bufs=2, space="PSUM") as tpsp:

                t_W = whp.tile([128, NK, 4 * H], F16, tag="Whh")
                nc.sync.dma_start(out=t_W, in_=d_WhhT[:].rearrange("(k p) n -> p k n", p=128))
                t_id = stp.tile([ACT_P, ACT_P], F32, tag="idm")
                nc.sync.dma_start(out=t_id, in_=d_id[:])

                # persistent ring tiles
                NXB = 3
                xq = [stp.tile([128, H], F32, tag=f"xq{i}", name=f"xq{i}") for i in range(NXB)]
                hT2 = [[stp.tile([128, 128], F16, tag=f"hT{i}s{s}", name=f"hT{i}s{s}")
                        for s in range(2)] for i in range(2)]
                c_r = [stp.tile([128, GQ], F32, tag=f"c{i}", name=f"c{i}") for i in range(2)]
                for i in range(NXB):
                    nc.vector.memset(xq[i], 0.0)
                for s in range(2):
                    nc.sync.dma_start(out=hT2[1][s], in_=d_h0T[:, s, :])
                nc.sync.dma_start(out=c_r[1], in_=d_c0[:])

                for t in range(t_steps):
                    cur, prv = t % 2, (t + 1) % 2
                    xcur = t % NXB
                    # per-step xW+bias: fwd at t (rows 0-7), bwd at T-1-t (rows 8-15)
                    for j in range(4):
                        nc.sync.dma_start(out=xq[xcur][32 * j:32 * j + RPC, :],
                                          in_=d_xw[t, j])
                        nc.sync.dma_start(out=xq[xcur][32 * j + RPC:32 * j + M, :],
                                          in_=d_xw[t_steps - 1 - t, j])

                    g_ps = gpsp.tile([128, 4 * H // 4], F32, tag="g")  # [128, 1024]
                    if t < 2:
                        nc.vector.memset(g_ps, 0.0)  # keep junk partitions finite
                    # chunk order: even chunks (hT half 0) first so the next
                    # step can begin once half 0 of h is transposed
                    K_ORDER = [0, 2, 4, 6, 1, 3, 5, 7]
                    for seg in range(NSEG):
                        cs = slice(seg * 512, (seg + 1) * 512)
                        # interleave col groups so the 4 PE column tiles
                        # stream concurrently
                        for ki, k in enumerate(K_ORDER):
                            for j in range(4):
                                wcols = slice(j * H + seg * 512, j * H + (seg + 1) * 512)
                                nc.tensor.matmul(
                                    out=g_ps[32 * j:32 * j + M, cs],
                                    lhsT=hT2[prv][k % 2][:, 32 * (k // 2):32 * (k // 2) + M],
                                    rhs=t_W[:, k, wcols],
                                    start=(ki == 0), stop=(ki == NK - 1),
                                    tile_position=(0, 32 * j))
                        if seg == 0:
                            t_gs = wkp.tile([128, 4 * H // 4], F32, tag="gs")
                            t_act = wkp.tile([128, 4 * H // 4], F32, tag="act")
                            nc.vector.tensor_add(out=t_gs[0:ACT_P, 0:512],
                                                 in0=g_ps[0:ACT_P, 0:512],
                                                 in1=xq[xcur][0:ACT_P, 0:512])
                            nc.scalar.activation(
                                out=t_act[0:ACT_P, 0:512], in_=t_gs[0:ACT_P, 0:512],
                                func=mybir.ActivationFunctionType.Sigmoid)
                    nc.vector.tensor_add(out=t_gs[0:ACT_P, 512:1024],
                                         in0=g_ps[0:ACT_P, 512:1024],
                                         in1=xq[xcur][0:ACT_P, 512:1024])
                    nc.scalar.activation(
                        out=t_act[0:ACT_P, 512:768], in_=t_gs[0:ACT_P, 512:768],
                        func=mybir.ActivationFunctionType.Sigmoid)
                    nc.scalar.activation(
                        out=t_act[0:ACT_P, 768:1024], in_=t_gs[0:ACT_P, 768:1024],
                        func=mybir.ActivationFunctionType.Tanh)

                    t_m1 = wkp.tile([128, GQ], F32, tag="m1")
                    t_m2 = wkp.tile([128, GQ], F32, tag="m2")
                    t_tc = wkp.tile([128, GQ], F32, tag="tc")
                    t_h = wkp.tile([128, GQ], F32, tag="h")
                    a = t_act[0:ACT_P]
                    for s in range(2):
                        hs = slice(s * 128, (s + 1) * 128)
                        nc.vector.tensor_mul(out=t_m1[0:ACT_P, hs],
                                             in0=a[:, s * 128:(s + 1) * 128],
                                             in1=a[:, 3 * GQ + s * 128:3 * GQ + (s + 1) * 128])
                        nc.vector.tensor_mul(out=t_m2[0:ACT_P, hs],
                                             in0=a[:, GQ + s * 128:GQ + (s + 1) * 128],
                                             in1=c_r[prv][0:ACT_P, hs])
                        nc.vector.tensor_add(out=c_r[cur][0:ACT_P, hs],
                                             in0=t_m1[0:ACT_P, hs], in1=t_m2[0:ACT_P, hs])
                        nc.scalar.activation(out=t_tc[0:ACT_P, hs], in_=c_r[cur][0:ACT_P, hs],
                                             func=mybir.ActivationFunctionType.Tanh)
                        nc.vector.tensor_mul(out=t_h[0:ACT_P, hs],
                                             in0=a[:, 2 * GQ + s * 128:2 * GQ + (s + 1) * 128],
                                             in1=t_tc[0:ACT_P, hs])
                        tp_ps = tpsp.tile([128, 128], F32, tag=f"tp{s}", name=f"tp{s}")
                        nc.tensor.transpose(
                            out=tp_ps[0:128, 0:ACT_P],
                            in_=t_h[0:ACT_P, hs],
                            identity=t_id)
                        nc.vector.tensor_copy(out=hT2[cur][s][:, 0:ACT_P],
                                              in_=tp_ps[:, 0:ACT_P])

                    # store forward rows (state rows 0-7 of each group)
                    for j in range(4):
                        nc.sync.dma_start(
                            out=d_hout[t, j],
                            in_=t_h[32 * j:32 * j + RPC, :])

                    if t == t_steps - 1:
                        nc.sync.dma_start(out=d_hfin[:], in_=t_h)
                        nc.sync.dma_start(out=d_cfin[:], in_=c_r[cur])
    nc.finalize()
    return nc


def _get_nc(t_steps: int):
    if t_steps not in _NC_CACHE:
        _NC_CACHE[t_steps] = _build(t_steps)
    return _NC_CACHE[t_steps]


def _prep_core_inputs(c, x, Wt_f16, WhhT_f16, bias_f32, idm, h0f, c0f, h0b, c0b,
                      t_steps):
    rows = slice(RPC * c, RPC * (c + 1))
    x_c = x[rows, :t_steps, :]                                # [8, T, I]
    xT = np.ascontiguousarray(
        x_c.reshape(RPC * t_steps, I).T).astype(np.float16)   # [I, R]

    hstate = np.concatenate([h0f[rows], h0b[rows]], axis=0)   # [16, H]
    cstate = np.concatenate([c0f[rows], c0b[rows]], axis=0)
    h0T = np.zeros((128, 2, 128), np.float16)
    c0 = np.zeros((128, GQ), np.float32)
    for j in range(4):
        for s in range(2):
            h0T[:, s, 32 * j:32 * j + M] = \
                hstate[:, 256 * j + 128 * s:256 * j + 128 * (s + 1)].T
        c0[32 * j:32 * j + M, :] = cstate[:, GQ * j:GQ * (j + 1)]
    return {"xT": xT, "Wt": Wt_f16, "WhhT": WhhT_f16, "bias": bias_f32,
            "idm": idm, "h0T": h0T, "c0": c0}


def kernel(x, Wih_f, Whh_f, bih_f, bhh_f, Wih_b, Whh_b, bih_b, bhh_b,
           h0f, c0f, h0b, c0b, t_steps=T, trace=False):
    global LAST_RESULTS
    x = np.asarray(x, np.float32)
    Wih_f = np.asarray(Wih_f, np.float32)
    Whh_f = np.asarray(Whh_f, np.float32)
    bias = np.asarray(bih_f, np.float32) + np.asarray(bhh_f, np.float32)
    h0f = np.asarray(h0f, np.float32); c0f = np.asarray(c0f, np.float32)
    h0b = np.asarray(h0b, np.float32); c0b = np.asarray(c0b, np.float32)

    Wt_f16 = np.ascontiguousarray(Wih_f.T[:, PERM]).astype(np.float16)
    WhhT_f16 = np.ascontiguousarray(Whh_f.T[:, PERM]).astype(np.float16)
    bias_f32 = bias[PERM].astype(np.float32).reshape(1, 4 * H)
    idm = np.eye(ACT_P, dtype=np.float32)

    nc = _get_nc(t_steps)
    in_maps = [
        _prep_core_inputs(c, x, Wt_f16, WhhT_f16, bias_f32, idm,
                          h0f, c0f, h0b, c0b, t_steps)
        for c in range(NCORES)
    ]
    res = run_bass_kernel_spmd(nc, in_maps, core_ids=list(range(NCORES)),
                               trace=trace)
    LAST_RESULTS = res

    out_f = np.empty((B, t_steps, H), np.float32)
    hf = np.empty((B, H), np.float32); hb = np.empty((B, H), np.float32)
    cf = np.empty((B, H), np.float32); cb = np.empty((B, H), np.float32)
    for c in range(NCORES):
        rows = slice(RPC * c, RPC * (c + 1))
        ho = res.results[c]["hout"]                    # [T, 4, 8, 256]
        out_f[rows] = ho.transpose(2, 0, 1, 3).reshape(RPC, t_steps, H)
        hfin = res.results[c]["hfin"].reshape(4, 32, GQ)
        cfin = res.results[c]["cfin"].reshape(4, 32, GQ)
        hstate = hfin[:, :M, :].transpose(1, 0, 2).reshape(M, H)
        cstate = cfin[:, :M, :].transpose(1, 0, 2).reshape(M, H)
        hf[rows] = hstate[:RPC]; hb[rows] = hstate[RPC:]
        cf[rows] = cstate[:RPC]; cb[rows] = cstate[RPC:]

    output = np.concatenate([out_f, out_f[::-1]], axis=-1)
    h_i = np.concatenate([hf, hb], axis=-1)
    c_i = np.concatenate([cf, cb], axis=-1)
    return output, (h_i, c_i)
